# revision 13
# baseline (speedup 1.0000x reference)
"""Trainium2 Bass kernel for EnhancedAttentionLayer (RoPE + ALiBi attention).

Sharding: 8 cores = 2 batches x 4 head-groups (3 heads each).
Each core computes, for its (b, heads) shard:
  qkv projections -> rope -> scores^T -> exp(+alibi bias) -> attn@v
  -> per-query normalization -> partial output projection (its heads' slice
  of the Wo contraction).
Host sums the 4 partial yT per batch (tensor-parallel "AllReduce") and
transposes back.

All heavy matmuls run as float32r (full PE rate). Softmax stability comes
from shifting scores by the constant alibi max (softmax-invariant), applied
as the per-partition bias of the Exp activation in [key, query] layout.
"""

import sys

if "/opt/trn_rl_repo" not in sys.path:
    sys.path.insert(0, "/opt/trn_rl_repo")

import numpy as np

import concourse.bass as bass
import concourse.bacc as bacc
import concourse.mybir as mybir
from concourse.tile import TileContext
from concourse.masks import make_identity

F32 = mybir.dt.float32
F32R = mybir.dt.float32r
EXP = mybir.ActivationFunctionType.Exp

B, S, D = 2, 2048, 1536
H, HD = 12, 128
HPC = 3            # heads per core
NCORES = 8
NCHUNK = D // 128  # 12 contraction chunks
NKT = S // 128     # 16 key tiles
ROPE_BASE = 10000.0


def _alibi_slopes(n):
    import math

    def pow2_slopes(m):
        start = 2.0 ** (-(2.0 ** (-(math.log2(m) - 3))))
        return [start * (start**i) for i in range(m)]

    if math.log2(n).is_integer():
        s = pow2_slopes(n)
    else:
        c = 2 ** math.floor(math.log2(n))
        s = pow2_slopes(c) + pow2_slopes(2 * c)[0::2][: n - c]
    return np.array(s, dtype=np.float32)


def build_program():
    nc = bacc.Bacc()

    xT = nc.dram_tensor("xT", [D, S], F32, kind="ExternalInput")
    Wqkv = nc.dram_tensor("Wqkv", [HPC, 3, NCHUNK, 128, 128], F32,
                          kind="ExternalInput")
    Wo_t = nc.dram_tensor("Wo_t", [HPC, NCHUNK, 128, 128], F32,
                          kind="ExternalInput")
    cosS = nc.dram_tensor("cosS", [128, S], F32, kind="ExternalInput")
    sinS = nc.dram_tensor("sinS", [128, S], F32, kind="ExternalInput")
    rbias = nc.dram_tensor("rbias", [128, HPC * NKT], F32, kind="ExternalInput")
    bqkv = nc.dram_tensor("bqkv", [128, HPC * 3], F32, kind="ExternalInput")
    bo_col = nc.dram_tensor("bo_col", [128, NCHUNK], F32, kind="ExternalInput")
    onesin = nc.dram_tensor("onesin", [128, 128], F32, kind="ExternalInput")

    yT = nc.dram_tensor("yT", [D, S], F32, kind="ExternalOutput")

    with TileContext(nc) as tc:
        with (
            tc.tile_pool(name="const", bufs=1) as constp,
            tc.tile_pool(name="xc", bufs=4) as xcp,
            tc.tile_pool(name="wt", bufs=6) as wtp,
            tc.tile_pool(name="qkv", bufs=2) as qkvp,
            tc.tile_pool(name="tmp", bufs=2) as tmpp,
            tc.tile_pool(name="pt", bufs=3) as ptp,
            tc.tile_pool(name="wb", bufs=2) as wbp,
            tc.tile_pool(name="rc", bufs=2) as rcp,
            tc.tile_pool(name="yst", bufs=3) as ystp,
            tc.tile_pool(name="outp", bufs=1) as outp,
            tc.tile_pool(name="ps", bufs=4, space="PSUM") as psp,
        ):
            cos_sb = constp.tile([128, S], F32, tag="cos")
            sin_sb = constp.tile([128, S], F32, tag="sin")
            rb_sb = constp.tile([128, HPC * NKT], F32, tag="rb")
            bqkv_sb = constp.tile([128, HPC * 3], F32, tag="bqkv")
            bo_sb = constp.tile([128, NCHUNK], F32, tag="bo")
            ones_col = constp.tile([128, 1], F32R, tag="ones")
            ones_row = constp.tile([1, 128], F32R, tag="onesr")
            ident = constp.tile([128, 128], F32, tag="ident")

            nc.sync.dma_start(out=cos_sb, in_=cosS[:])
            nc.sync.dma_start(out=sin_sb, in_=sinS[:])
            nc.sync.dma_start(out=rb_sb, in_=rbias[:])
            nc.sync.dma_start(out=bqkv_sb, in_=bqkv[:])
            nc.sync.dma_start(out=bo_sb, in_=bo_col[:])
            nc.sync.dma_start(out=ones_col, in_=onesin[:, 0:1].bitcast(F32R))
            nc.sync.dma_start(out=ones_row, in_=onesin[0:1, :].bitcast(F32R))
            make_identity(nc, ident)

            out_sb = [outp.tile([128, S], F32R, tag=f"out{h}", name=f"out{h}")
                      for h in range(HPC)]

            for h in range(HPC):
                q_sb = qkvp.tile([128, S], F32R, tag="q")
                k_sb = qkvp.tile([128, S], F32R, tag="k")
                v_sb = qkvp.tile([128, S], F32R, tag="v")

                for sh in range(2):
                    ssl = slice(sh * 1024, (sh + 1) * 1024)
                    proj_ps = [psp.tile([128, 1024], F32, tag="ps",
                                        name=f"proj{h}_{sh}_{pi}")
                               for pi in range(3)]
                    for c in range(NCHUNK):
                        x_t = xcp.tile([128, 1024], F32R, tag="xc")
                        nc.sync.dma_start(
                            out=x_t, in_=xT[c * 128:(c + 1) * 128, ssl].bitcast(F32R))
                        for pi in range(3):
                            w_t = wtp.tile([128, 128], F32R, tag="w")
                            nc.sync.dma_start(out=w_t, in_=Wqkv[h, pi, c, :, :].bitcast(F32R))
                            for sl in range(2):
                                nc.tensor.matmul(
                                    proj_ps[pi][:, sl * 512:(sl + 1) * 512],
                                    w_t,
                                    x_t[:, sl * 512:(sl + 1) * 512],
                                    start=(c == 0), stop=(c == NCHUNK - 1))

                    # q, k: bias + rope.  v: bias only, then transpose blocks.
                    for pi, dst in ((0, q_sb), (1, k_sb)):
                        raw = tmpp.tile([128, 1024], F32, tag="raw")
                        nc.vector.tensor_scalar_add(
                            raw, proj_ps[pi], bqkv_sb[:, h * 3 + pi:h * 3 + pi + 1])
                        sw = tmpp.tile([128, 1024], F32, tag="sw")
                        nc.gpsimd.tensor_copy(sw[0:64, :], raw[64:128, :])
                        nc.gpsimd.tensor_copy(sw[64:128, :], raw[0:64, :])
                        t1 = tmpp.tile([128, 1024], F32, tag="t1")
                        nc.vector.tensor_mul(t1, raw, cos_sb[:, ssl])
                        nc.vector.tensor_mul(dst[:, ssl], sw, sin_sb[:, ssl])
                        nc.vector.tensor_add(dst[:, ssl], dst[:, ssl].bitcast(F32), t1)

                    vt = tmpp.tile([128, 1024], F32, tag="raw")
                    nc.vector.tensor_scalar_add(
                        vt, proj_ps[2], bqkv_sb[:, h * 3 + 2:h * 3 + 3])
                    # transpose [hd, seq]->[seq, hd] in 128-blocks, 4 per psum
                    for grp in range(2):
                        tr_ps = psp.tile([128, 1024], F32, tag="ps")
                        for j in range(4):
                            nc.tensor.transpose(
                                tr_ps[:, j * 128:(j + 1) * 128],
                                vt[:, (grp * 4 + j) * 128:(grp * 4 + j + 1) * 128],
                                ident)
                        nc.vector.tensor_copy(
                            v_sb[:, sh * 1024 + grp * 512: sh * 1024 + (grp + 1) * 512],
                            tr_ps[:, 0:512])

                # attention for this head
                for qg in range(2):
                    qsl = slice(qg * 1024, (qg + 1) * 1024)
                    av_ps = psp.tile([128, 1024], F32, tag="ps")
                    den_ps = psp.tile([1, 1024], F32, tag="ps")
                    for kt in range(NKT):
                        sc_ps = psp.tile([128, 1024], F32, tag="ps")
                        for sl in range(2):
                            nc.tensor.matmul(
                                sc_ps[:, sl * 512:(sl + 1) * 512],
                                k_sb[:, kt * 128:(kt + 1) * 128],
                                q_sb[:, qg * 1024 + sl * 512:
                                     qg * 1024 + (sl + 1) * 512],
                                start=True, stop=True)
                        pt_sb = ptp.tile([128, 1024], F32R, tag="pt")
                        nc.scalar.activation(
                            pt_sb, sc_ps, EXP,
                            bias=rb_sb[:, h * NKT + kt:h * NKT + kt + 1])
                        for sl in range(2):
                            psl = slice(sl * 512, (sl + 1) * 512)
                            nc.tensor.matmul(
                                av_ps[:, psl],
                                v_sb[:, kt * 128:(kt + 1) * 128],
                                pt_sb[:, psl],
                                start=(kt == 0), stop=(kt == NKT - 1))
                            nc.tensor.matmul(
                                den_ps[:, psl],
                                ones_col,
                                pt_sb[:, psl],
                                start=(kt == 0), stop=(kt == NKT - 1))
                    rc_sb = rcp.tile([1, 1024], F32R, tag="rc")
                    with nc.allow_low_precision(
                            reason="f32r is 4-byte; PE rounds anyway"):
                        nc.vector.reciprocal(rc_sb, den_ps)
                    w_ps = psp.tile([128, 1024], F32, tag="ps", name="w_ps")
                    for sl in range(2):
                        psl = slice(sl * 512, (sl + 1) * 512)
                        nc.tensor.matmul(w_ps[:, psl], ones_row,
                                         rc_sb[:, psl],
                                         start=True, stop=True)
                    w_sb = wbp.tile([128, 1024], F32, tag="wb")
                    nc.scalar.copy(w_sb, w_ps)
                    nc.vector.tensor_mul(out_sb[h][:, qsl], av_ps, w_sb)

            # output projection: yT[co] = sum_h Wo_t[h,co].T @ out_sb[h]
            for co in range(NCHUNK):
                wo_t = []
                for hi in range(HPC):
                    w_t = wtp.tile([128, 128], F32R, tag="w")
                    nc.sync.dma_start(out=w_t, in_=Wo_t[hi, co, :, :].bitcast(F32R))
                    wo_t.append(w_t)
                for qh in range(2):
                    y_ps = psp.tile([128, 1024], F32, tag="ps")
                    for hi in range(HPC):
                        for sl in range(2):
                            off = qh * 1024 + sl * 512
                            nc.tensor.matmul(
                                y_ps[:, sl * 512:(sl + 1) * 512],
                                wo_t[hi],
                                out_sb[hi][:, off:off + 512],
                                start=(hi == 0), stop=(hi == HPC - 1))
                    y_sb = ystp.tile([128, 1024], F32, tag="y")
                    nc.vector.tensor_scalar_add(y_sb, y_ps, bo_sb[:, co:co + 1])
                    nc.sync.dma_start(
                        out=yT[co * 128:(co + 1) * 128,
                               qh * 1024:(qh + 1) * 1024],
                        in_=y_sb)
    nc.compile()
    return nc


def make_inputs(x, Wq, bq, Wk, bk, Wv, bv, Wo, bo):
    """Build the per-core input maps (host-side sharding)."""
    x = np.ascontiguousarray(np.asarray(x, dtype=np.float32))
    Wq, Wk, Wv, Wo = (np.asarray(w, dtype=np.float32) for w in (Wq, Wk, Wv, Wo))
    bq, bk, bv, bo = (np.asarray(b, dtype=np.float32) for b in (bq, bk, bv, bo))

    perm = np.concatenate([np.arange(0, HD, 2), np.arange(1, HD, 2)])
    scale_q = float(HD) ** -0.25  # sqrt of attention scale, folded into tables

    inv_freq = 1.0 / (ROPE_BASE ** (np.arange(0, HD, 2, dtype=np.float32) / HD))
    t = np.arange(S, dtype=np.float32)
    freqs = np.outer(inv_freq, t)  # [64, S]
    cos64 = np.cos(freqs).astype(np.float32) * scale_q
    sin64 = np.sin(freqs).astype(np.float32) * scale_q
    cosS = np.concatenate([cos64, cos64], axis=0)          # [128, S]
    sinS = np.concatenate([-sin64, sin64], axis=0)         # [128, S]

    slopes = _alibi_slopes(H)

    xT = [np.ascontiguousarray(x[b].T) for b in range(B)]

    in_maps = []
    for c in range(NCORES):
        b = c // 4
        heads = [HPC * (c % 4) + j for j in range(HPC)]

        wqkv = np.empty((HPC, 3, NCHUNK, 128, 128), np.float32)
        bq_cols = np.empty((128, HPC * 3), np.float32)
        for hi, h in enumerate(heads):
            rows = h * HD + perm
            for pi, (W, bias) in enumerate(((Wq, bq), (Wk, bk), (Wv, bv))):
                r = rows if pi < 2 else np.arange(h * HD, (h + 1) * HD)
                Wh = W[r, :]  # [128, 1536]
                wqkv[hi, pi] = Wh.reshape(128, NCHUNK, 128).transpose(1, 2, 0)
                bq_cols[:, hi * 3 + pi] = bias[r]

        wo_t = np.empty((HPC, NCHUNK, 128, 128), np.float32)
        for hi, h in enumerate(heads):
            blk = Wo[:, h * HD:(h + 1) * HD]  # [1536, 128]
            wo_t[hi] = blk.reshape(NCHUNK, 128, 128).transpose(0, 2, 1)

        rb = np.empty((128, HPC * NKT), np.float32)
        for hi, h in enumerate(heads):
            r = slopes[h] * (np.arange(S, dtype=np.float32) - (S - 1))
            rb[:, hi * NKT:(hi + 1) * NKT] = r.reshape(NKT, 128).T

        bo_cols = (bo.reshape(NCHUNK, 128).T if c % 4 == 0
                   else np.zeros((128, NCHUNK), np.float32))

        in_maps.append({
            "xT": xT[b],
            "Wqkv": wqkv,
            "Wo_t": np.ascontiguousarray(wo_t),
            "cosS": cosS,
            "sinS": sinS,
            "rbias": rb,
            "bqkv": bq_cols,
            "bo_col": np.ascontiguousarray(bo_cols),
            "onesin": np.ones((128, 128), np.float32),
        })
    return in_maps


def gather_output(results):
    y = np.zeros((B, S, D), np.float32)
    for c, res in enumerate(results):
        y[c // 4] += res["yT"].T
    return y


_CACHED_NC = None


def kernel(**inputs):
    global _CACHED_NC
    from concourse.bass_utils import run_bass_kernel_spmd

    if _CACHED_NC is None:
        _CACHED_NC = build_program()
    in_maps = make_inputs(**inputs)
    res = run_bass_kernel_spmd(_CACHED_NC, in_maps, list(range(NCORES)))
    return gather_output(res.results)


if __name__ == "__main__":
    import jax
    sys.path.insert(0, "/root/problem")
    import reference

    inputs = {k: np.asarray(v) for k, v in reference.setup_inputs().items()}
    expected = np.asarray(reference.reference(**inputs))
    actual = kernel(**inputs)
    err = np.abs(actual - expected).max()
    rel = err / np.abs(expected).max()
    print(f"abs max err: {err:.6e}  rel: {rel:.6e}")
